# revision 1
# baseline (speedup 1.0000x reference)
"""Trainium2 Bass kernel for the Mahalanobis loss:

    out = mean_b( sqrt( delta[b] @ S_inv @ delta[b] ) ),  delta = original - reconstruction

Full shapes: original/reconstruction [8192, 2048] f32, S_inv [2048, 2048] f32.

Strategy (data-parallel over batch, 8 NeuronCores):
  - Core i handles rows [i*1024, (i+1)*1024). S_inv replicated.
  - Per core: delta computed on DVE (f32 sub -> bf16 out), transposed to
    [d, b] layout via DMA-transpose (bf16), S_inv cast to bf16 (ACT).
  - Y = delta @ S_inv as 128x128 stationary (delta^T tiles) x [128, 512]
    moving (S) bf16 matmuls accumulated f32 in PSUM over 16 K-blocks.
  - q[b] = rowsum(delta_bf16 * Y) fused in one DVE tensor_tensor_reduce per
    (b_tile, e_chunk), chain-accumulated into q_all[:, b_tile].
  - Per-core output: q_out [128, 8] f32 (q for its 1024 rows).
  - Host: concat shards, sqrt, mean  (exact f64 host math, cast to f32).

Numerics: bf16 matmul with f32 accumulation gives ~5e-5 relative error on the
final scalar (validated against f64 numpy).
"""

import numpy as np

P = 128
B_FULL, D = 8192, 2048
N_CORES = 8
B_SH = B_FULL // N_CORES  # 1024
EC = 512                  # matmul moving free dim / PSUM bank (f32)

_CACHED = {}


def _build(b_sh=B_SH, d=D, loop=1):
    import contextlib

    import concourse.tile as tile
    from concourse import bacc, mybir

    NB = b_sh // P   # batch tiles per core
    NJ = d // P      # contraction K-blocks
    NE = d // EC     # e-chunks (output columns / 512)

    # Bacc (not raw Bass): its compile() legalizes semaphore waits
    # (move_matmul_waits_to_ldweights + generate_event_semaphores) — TRN2
    # instructions can embed only ONE sync wait.
    nc = bacc.Bacc("TRN2", target_bir_lowering=False)
    f32 = mybir.dt.float32
    bf16 = mybir.dt.bfloat16

    orig = nc.dram_tensor("orig", [b_sh, d], f32, kind="ExternalInput")
    recon = nc.dram_tensor("recon", [b_sh, d], f32, kind="ExternalInput")
    s_inv = nc.dram_tensor("s_inv", [d, d], f32, kind="ExternalInput")
    q_out = nc.dram_tensor("q_out", [P, NB], f32, kind="ExternalOutput")

    with tile.TileContext(nc) as tc:
        with (
            tc.tile_pool(name="io", bufs=3) as io_pool,
            tc.tile_pool(name="sstage", bufs=8) as s_stage,
            tc.tile_pool(name="sbf", bufs=1) as s_pool,
            tc.tile_pool(name="dbf", bufs=1) as d_pool,
            tc.tile_pool(name="dT", bufs=1) as dT_pool,
            tc.tile_pool(name="scr", bufs=2) as scr_pool,
            tc.tile_pool(name="qp", bufs=1) as q_pool,
            tc.tile_pool(name="psum", bufs=8, space="PSUM") as psum_pool,
            tc.For_i(0, loop, 1) if loop > 1 else contextlib.nullcontext(),
        ):
            q_all = q_pool.tile([P, NB], f32, name="q_all", tag="q_all")
            q_part = q_pool.tile([P, NB, NE], f32, name="q_part", tag="q_part")
            delta_bf = [None] * NB
            deltaT = [None] * NB
            s_bf = [[None] * NE for _ in range(NJ)]

            def emit_delta(t):
                # delta pipeline for batch tile t.
                # Plain loads go on the ACT (scalar) HWDGE queue so the SP
                # queue carries only transposes: a transpose waits on the DVE
                # subtract, and an in-order DMA queue would stall every later
                # load behind that wait.
                o_t = io_pool.tile([P, d], f32, name=f"o_{t}", tag="o")
                nc.scalar.dma_start(o_t[:], orig[t * P:(t + 1) * P, :])
                r_t = io_pool.tile([P, d], f32, name=f"r_{t}", tag="r")
                nc.scalar.dma_start(r_t[:], recon[t * P:(t + 1) * P, :])
                db = d_pool.tile([P, d], bf16, name=f"dbf_{t}", tag=f"dbf_{t}")
                nc.vector.tensor_sub(db[:], o_t[:], r_t[:])
                dT = dT_pool.tile([P, NJ, P], bf16, name=f"dT_{t}",
                                  tag=f"dT_{t}")
                # dT[p, j, b] = db[b, j*128 + p]  (verified in CoreSim)
                nc.sync.dma_start(dT[:], db[:], transpose=True)
                delta_bf[t] = db
                deltaT[t] = dT

            def emit_s_chunk(e):
                for j in range(NJ):
                    sf = s_stage.tile([P, EC], f32, name=f"sf_{j}_{e}",
                                      tag="sf")
                    nc.sync.dma_start(
                        sf[:], s_inv[j * P:(j + 1) * P, e * EC:(e + 1) * EC])
                    sb = s_pool.tile([P, EC], bf16, name=f"s_{j}_{e}",
                                     tag=f"s_{j}_{e}")
                    nc.scalar.copy(sb[:], sf[:])
                    s_bf[j][e] = sb

            # Emission order == desired load order: delta tiles (2 MiB each)
            # and S e-chunks (4 MiB each) interleaved so loaded-deltas ≈
            # 2 × loaded-S-chunks, which maximizes ready matmul cells per
            # loaded byte. Matmul cells are emitted in data-ready "waves"
            # matching that order, so the PE never waits on far-future loads.
            if NB == 8 and NE == 4:
                load_order = [("d", 0), ("S", 0), ("d", 1), ("S", 1),
                              ("d", 2), ("d", 3), ("S", 2), ("d", 4),
                              ("d", 5), ("S", 3), ("d", 6), ("d", 7)]
            else:
                load_order = []
                for i in range(max(NB, NE)):
                    if i < NB:
                        load_order.append(("d", i))
                    if i < NE:
                        load_order.append(("S", i))
            have_d, have_s = set(), set()
            waves = []
            for kind, idx in load_order:
                if kind == "d":
                    emit_delta(idx)
                    have_d.add(idx)
                    waves.append([(idx, e) for e in sorted(have_s)])
                else:
                    emit_s_chunk(idx)
                    have_s.add(idx)
                    waves.append([(t, idx) for t in sorted(have_d)])

            def emit_cell(t, e):
                ps = psum_pool.tile([P, EC], f32, name=f"ps_{e}_{t}", tag="ps")
                for j in range(NJ):
                    nc.tensor.matmul(
                        ps[:],
                        deltaT[t][:, j, :],
                        s_bf[j][e][:],
                        start=(j == 0),
                        stop=(j == NJ - 1),
                    )
                return ps

            for wave in waves:
                for (t, e) in wave:
                    ps = emit_cell(t, e)
                    # q-partial: product then row-reduce (two plain DVE ops;
                    # tensor_tensor_reduce faults the device on this runtime)
                    scr = scr_pool.tile([P, EC], f32, name=f"scr_{e}_{t}",
                                        tag="scr")
                    nc.vector.tensor_tensor(
                        scr[:], ps[:], delta_bf[t][:, e * EC:(e + 1) * EC],
                        mybir.AluOpType.mult)
                    nc.vector.tensor_reduce(
                        out=q_part[:, t, e:e + 1], in_=scr[:],
                        axis=mybir.AxisListType.X, op=mybir.AluOpType.add)

            nc.vector.tensor_reduce(out=q_all[:, :, None], in_=q_part[:],
                                    axis=mybir.AxisListType.X,
                                    op=mybir.AluOpType.add)
            nc.sync.dma_start(q_out[:], q_all[:])

    nc.compile()
    return nc


def _get_nc():
    if "nc" not in _CACHED:
        _CACHED["nc"] = _build()
    return _CACHED["nc"]


def kernel(original: np.ndarray, reconstruction: np.ndarray,
           S_inv: np.ndarray) -> np.ndarray:
    from concourse import bass_utils

    nc = _get_nc()
    s_full = np.ascontiguousarray(np.asarray(S_inv, dtype=np.float32))
    in_maps = []
    for i in range(N_CORES):
        sl = slice(i * B_SH, (i + 1) * B_SH)
        in_maps.append({
            "orig": np.ascontiguousarray(np.asarray(original[sl], np.float32)),
            "recon": np.ascontiguousarray(
                np.asarray(reconstruction[sl], np.float32)),
            "s_inv": s_full,
        })

    res = bass_utils.run_bass_kernel_spmd(
        nc, in_maps, core_ids=list(range(N_CORES)),
        trace=_CACHED.get("trace", False),
    )
    _CACHED["last_results"] = res

    q = np.concatenate(
        [np.asarray(r["q_out"]).T.reshape(-1) for r in res.results])
    out = np.sqrt(q.astype(np.float64)).mean()
    return np.float32(out)



# revision 3
# speedup vs baseline: 1.6696x; 1.6696x over previous
"""Trainium2 Bass kernel for the Mahalanobis loss:

    out = mean_b( sqrt( delta[b] @ S_inv @ delta[b] ) ),  delta = original - reconstruction

Full shapes: original/reconstruction [8192, 2048] f32, S_inv [2048, 2048] f32.
Data-parallel over batch on 8 NeuronCores: core i handles rows [i*1024,(i+1)*1024).

Math: S_inv is symmetric, so  q[b] = 2 * delta[b] @ U' @ delta[b]  with
U' = strict_upper(S) + diag(S)/2.  Only the 136 upper-triangular 128x128
blocks of S are uploaded (8.5 MiB instead of 16) and matmul'd (53% of the
dense FLOPs).  Per-core HBM traffic: 16 MiB x (orig|recon transposed,
block-packed) + 8.5 MiB S-upper = 24.5 MiB, the binding resource.

Device kernel (per core):
  - x_t [16,128,2,2,512] f32: host-transposed orig/recon packed so each
    d-block is one contiguous 1 MiB DMA (measured ~1.4x faster than strided
    half-loads).  deltaT = orig - recon computed per batch-half straight to
    fp8e4 on DVE (h0) and Pool (h1) -- delta in [d, b] layout, no on-device
    transpose.
  - S blocks cast f32 -> fp8e4 with scale 16 on ACT (off-diag ~N(0,1/45)
    would sit in fp8 subnormal range unscaled); 1/16 folded into the final
    reduction vector.  Diagonal blocks masked on-device (strict upper +
    diag/2) via gpsimd affine_select triangle mask.
  - Matmuls in FP8 DoubleRow: stationary [128, 2, 128] = two adjacent
    d-blocks of an S column-block, moving [128, 2, 512] = the matching two
    delta blocks, 0.5 cycles/row => PE ~25 us, fully DMA-bound.
    Yt[e,b] (e-block c) accumulates over j-pairs in PSUM [128,512] halves.
  - The same fp8 delta feeds the matmul and the final elementwise product,
    so q is the exact quadratic form of the perturbed inputs (error
    ~2 eps^T S delta ~ 0.06% measured, tolerance 2e-2).
  - Per column close: prod = deltaT_c .* Yt_c (DVE, bf16 out), then its
    row-sums accumulate into one persistent PSUM bank via a (2/16)-vector
    matmul (partitions 0/32 hold the two batch halves).
  - Tail: ACT sqrt with accum_out fuses sqrt+sum -> per-core [1,2] output.

Schedule: j-major over arriving delta blocks for the two BIG columns (14,15,
held in 4 PSUM banks); small columns 0..13 stream through the remaining
banks, paced to cover the late delta phase; S chunks preloaded so the x
stream is always the DMA tail; the last DMA (delta_15) unlocks only the
final DoubleRow pair of column 15.  Host: sum 16 half-sums / 8192.

Measured (hardware-loop amortized, 8 cores): ~93 us/iter vs ~198 us for the
bf16 dense-block baseline kernel -- 2.1x.  DMA floor for the 24.5 MiB at
measured ~320 GB/s effective is ~77 us.
"""

import numpy as np

P = 128
B_FULL, D = 8192, 2048
N_CORES = 8
B_SH = B_FULL // N_CORES    # 1024
NJ = D // P                 # 16 d/e blocks
NBLK = NJ * (NJ + 1) // 2   # 136 upper blocks
S_COLS = P * NBLK           # 17408
S_BOFF = [i * (i + 1) // 2 for i in range(NJ + 1)]  # block index offsets

BIG_COLS = [14, 15]
SMALL_COLS = [c for c in range(NJ) if c not in BIG_COLS]
N_SMALL_CELLS = sum(c + 1 for c in SMALL_COLS)   # 91
S_SCALE = 16.0

_CACHED = {}


def _build(b_sh=B_SH, d=D, loop=1):
    import contextlib

    import concourse.tile as tile
    from concourse import bacc, mybir

    nc = bacc.Bacc("TRN2", target_bir_lowering=False)
    f32 = mybir.dt.float32
    f32r = mybir.dt.float32r
    fp8 = mybir.dt.float8e4
    bf16 = mybir.dt.bfloat16
    DR = mybir.MatmulPerfMode.DoubleRow

    # [d-block, p, (orig|recon), batch-half, 512]
    x_t = nc.dram_tensor("x_t", [NJ, P, 2, 2, 512], f32,
                         kind="ExternalInput")
    s_pack = nc.dram_tensor("s_pack", [P, S_COLS], f32, kind="ExternalInput")
    q_out = nc.dram_tensor("q_out", [1, 2], f32, kind="ExternalOutput")

    with tile.TileContext(nc) as tc:
        with (
            tc.tile_pool(name="io", bufs=8) as io_pool,
            tc.tile_pool(name="sstage", bufs=3) as s_stage,
            tc.tile_pool(name="sbf", bufs=1) as s_pool,
            tc.tile_pool(name="dT", bufs=1) as dT_pool,
            tc.tile_pool(name="mkb", bufs=1) as mkb_pool,
            tc.tile_pool(name="mk", bufs=4) as mk_pool,
            tc.tile_pool(name="pr", bufs=4) as pr_pool,
            tc.tile_pool(name="accp", bufs=1) as acc_pool,
            tc.tile_pool(name="cst", bufs=1) as cst_pool,
            tc.tile_pool(name="tail", bufs=1) as tail_pool,
            tc.tile_pool(name="psq", bufs=1, space="PSUM") as psq_pool,
            tc.tile_pool(name="psbig", bufs=1, space="PSUM") as psb_pool,
            tc.tile_pool(name="pssm", bufs=3, space="PSUM") as pss_pool,
            tc.For_i(0, loop, 1) if loop > 1 else contextlib.nullcontext(),
        ):
            # --- constants ---
            tri = cst_pool.tile([P, P], f32, name="tri", tag="tri")
            nc.gpsimd.memset(tri[:], 1.0)
            nc.gpsimd.affine_select(
                out=tri[:], in_=tri[:], compare_op=mybir.AluOpType.is_ge,
                fill=0.5, base=-1, channel_multiplier=-1, pattern=[[1, P]])
            nc.gpsimd.affine_select(
                out=tri[:], in_=tri[:], compare_op=mybir.AluOpType.is_ge,
                fill=0.0, base=0, channel_multiplier=-1, pattern=[[1, P]])
            twos = cst_pool.tile([P, 1], bf16, name="twos", tag="twos")
            nc.vector.memset(twos[:], 2.0 / S_SCALE)

            # S blocks as [P, block, 128] fp8 (scaled by 16)
            warm = cst_pool.tile([1, 1], f32, name="warm", tag="warm")
            nc.scalar.sqrt(warm[:], tri[0:1, 0:1])

            s8 = s_pool.tile([P, NBLK, P], fp8, name="s8", tag="s8")
            qps2 = psq_pool.tile([64, 512], f32, name="qps2", tag="qps2")
            qps = [qps2[32 * h:32 * h + 1, :] for h in range(2)]
            # delta pair tiles [P, ko, h, 512]: ko = which block of the pair
            dpair = [dT_pool.tile([P, 2, 2, 512], fp8, name=f"dT_{m}",
                                  tag=f"dT_{m}") for m in range(NJ // 2)]
            masks = {}

            def emit_block(j):
                # one contiguous 1 MiB DMA per d-block (439 GB/s measured vs
                # 250 GB/s for strided half-loads), then per-half subs on two
                # engines
                xt = io_pool.tile([P, 2, 2, 512], f32, name=f"x_{j}",
                                  tag="io")
                nc.sync.dma_start(xt[:], x_t[j])
                for h in range(2):
                    eng = nc.vector if h == 0 else nc.gpsimd
                    eng.tensor_sub(dpair[j // 2][:, j % 2, h, :],
                                   xt[:, 0, h, :], xt[:, 1, h, :])

            def emit_s_chunk(c, big=False):
                b0, b1 = S_BOFF[c], S_BOFF[c + 1]
                st = s_stage.tile([P, NJ, P], f32, name=f"sg_{c}", tag="sg")
                nc.sync.dma_start(st[:, 0:b1 - b0, :],
                                  s_pack[:, b0 * P:b1 * P])
                nc.scalar.mul(s8[:, b0:b1, :], st[:, 0:b1 - b0, :], S_SCALE)
                pool = mkb_pool if big else mk_pool
                # masked diag block (strict upper + diag/2); for odd c the
                # DoubleRow pair needs [block c-1 | masked c] side by side.
                nko = 2 if c % 2 == 1 else 1
                mk = pool.tile([P, nko, P], fp8, name=f"mk_{c}",
                               tag=f"mk_{c}" if big else "mk")
                if nko == 2:
                    nc.scalar.copy(mk[:, 0, :], s8[:, S_BOFF[c] + c - 1, :])
                nc.gpsimd.tensor_tensor(
                    mk[:, nko - 1, :], s8[:, S_BOFF[c] + c, :], tri[:],
                    mybir.AluOpType.mult)
                masks[c] = mk

            def emit_cells(j0, c, ph):
                """pair-cell covering j0 (even) and j0+1 if <= c; or the
                masked single for even c."""
                first = (j0 == 0)
                if j0 + 1 <= c:   # DoubleRow pair (j0, j0+1)
                    last = (j0 + 1 == c)
                    lhsT = masks[c][:, :, :] if last else \
                        s8[:, S_BOFF[c] + j0:S_BOFF[c] + j0 + 2, :]
                    for h in range(2):
                        nc.tensor.matmul(
                            ph[h][:], lhsT, dpair[j0 // 2][:, :, h, :],
                            start=first, stop=last, perf_mode=DR)
                else:             # single masked diag (c even, j0 == c)
                    for h in range(2):
                        nc.tensor.matmul(
                            ph[h][:], masks[c][:, 0, :],
                            dpair[j0 // 2][:, j0 % 2, h, :],
                            start=first, stop=True)

            n_closed = [0]

            def emit_prod_acc(c, ph):
                # prod = delta_c .* Yt_c (DVE), then accumulate its row-sums
                # into the persistent q banks via a ones(=2/16)-matmul (PE).
                for h in range(2):
                    dlast = dpair[c // 2][:, c % 2, h, :]
                    prod = pr_pool.tile([P, 512], bf16,
                                        name=f"pr_{c}_{h}", tag="pr")
                    nc.vector.tensor_tensor(prod[:], ph[h][:], dlast,
                                            mybir.AluOpType.mult)
                    nc.tensor.matmul(qps[h], twos[:],
                                     prod[:],
                                     start=(n_closed[0] == 0),
                                     stop=(n_closed[0] == NJ - 1),
                                     skip_group_check=True)
                n_closed[0] += 1

            # --- schedule ---
            emit_block(0)
            emit_block(1)
            big_ph = {}
            for c in BIG_COLS:
                emit_s_chunk(c, big=True)
                big_ph[c] = [psb_pool.tile([P, 512], f32, name=f"psb_{c}_{h}",
                                           tag=f"psb_{c}_{h}")
                             for h in range(2)]
            emit_block(2)

            smalls = list(SMALL_COLS)
            to_load = list(SMALL_COLS)
            small_done = 0
            for j in range(NJ):
                if j + 3 < NJ:
                    emit_block(j + 3)
                # preload small-column S chunks well ahead of their cells so
                # the x blocks are always the stream's tail
                while to_load and to_load[0] <= j + 4:
                    emit_s_chunk(to_load.pop(0))
                for c in BIG_COLS:
                    if j <= c and (j % 2 == 1 or j == c):
                        emit_cells(j - 1 if j % 2 == 1 else j, c, big_ph[c])
                        if j == c:
                            emit_prod_acc(c, big_ph[c])
                cap = (N_SMALL_CELLS * (j + 1) + 11) // 12
                while smalls and smalls[0] <= j and \
                        small_done + smalls[0] + 1 <= cap:
                    c = smalls.pop(0)
                    ph = [pss_pool.tile([P, 512], f32, name=f"ps_{c}_{h}",
                                        tag="ps") for h in range(2)]
                    for j0 in range(0, c + 1, 2):
                        emit_cells(j0, c, ph)
                    emit_prod_acc(c, ph)
                    small_done += c + 1

            # --- tail: q is already in qps; fused sqrt+sum per half ---
            red = tail_pool.tile([1, 2], f32, name="red", tag="red")
            sq = tail_pool.tile([1, b_sh], f32, name="sq", tag="sq")
            for h in range(2):
                nc.scalar.activation(
                    out=sq[:, h * 512:(h + 1) * 512], in_=qps[h],
                    func=mybir.ActivationFunctionType.Sqrt,
                    accum_out=red[:, h:h + 1])
            nc.sync.dma_start(q_out[:], red[:])

    nc.compile()
    return nc


def _get_nc():
    if "nc" not in _CACHED:
        _CACHED["nc"] = _build()
    return _CACHED["nc"]


def make_in_maps(original, reconstruction, S_inv):
    """Host-side sharding/packing (pure slicing + layout rearrangement)."""
    s = np.asarray(S_inv, dtype=np.float32)
    s_pack = np.ascontiguousarray(np.concatenate(
        [s[j * P:(j + 1) * P, i * P:(i + 1) * P]
         for i in range(NJ) for j in range(i + 1)], axis=1))

    in_maps = []
    for i in range(N_CORES):
        sl = slice(i * B_SH, (i + 1) * B_SH)
        x = np.empty((D, 2 * B_SH), np.float32)
        x[:, 0:B_SH] = np.asarray(original[sl], np.float32).T
        x[:, B_SH:] = np.asarray(reconstruction[sl], np.float32).T
        in_maps.append({"x_t": x.reshape(NJ, P, 2, 2, 512), "s_pack": s_pack})
    return in_maps


def kernel(original: np.ndarray, reconstruction: np.ndarray,
           S_inv: np.ndarray) -> np.ndarray:
    from concourse import bass_utils

    nc = _get_nc()
    in_maps = make_in_maps(original, reconstruction, S_inv)
    res = bass_utils.run_bass_kernel_spmd(
        nc, in_maps, core_ids=list(range(N_CORES)),
        trace=_CACHED.get("trace", False),
    )
    _CACHED["last_results"] = res

    total = sum(float(np.asarray(r["q_out"]).sum()) for r in res.results)
    return np.float32(total / B_FULL)


# revision 4
# speedup vs baseline: 1.7348x; 1.0391x over previous
"""Trainium2 Bass kernel for the Mahalanobis loss:

    out = mean_b( sqrt( delta[b] @ S_inv @ delta[b] ) ),  delta = original - reconstruction

Full shapes: original/reconstruction [8192, 2048] f32, S_inv [2048, 2048] f32.
Data-parallel over batch on 8 NeuronCores: core i handles rows [i*1024,(i+1)*1024).

Math: S_inv is symmetric, so  q[b] = 2 * delta[b] @ U' @ delta[b]  with
U' = strict_upper(S) + diag(S)/2.  Only the 136 upper-triangular 128x128
blocks of S are uploaded (8.5 MiB instead of 16) and matmul'd (53% of the
dense FLOPs).  Per-core HBM traffic: 16 MiB x (orig|recon transposed,
block-packed) + 8.5 MiB S-upper = 24.5 MiB, the binding resource.

Device kernel (per core):
  - x_t [16,128,2,2,512] f32: host-transposed orig/recon packed so each
    d-block is one contiguous 1 MiB DMA (measured ~1.4x faster than strided
    half-loads).  deltaT = orig - recon computed per batch-half straight to
    fp8e4 on DVE (h0) and Pool (h1) -- delta in [d, b] layout, no on-device
    transpose.
  - S blocks cast f32 -> fp8e4 with scale 16 on ACT (off-diag ~N(0,1/45)
    would sit in fp8 subnormal range unscaled); 1/16 folded into the final
    reduction vector.  Diagonal blocks masked on-device (strict upper +
    diag/2) via gpsimd affine_select triangle mask.
  - Matmuls in FP8 DoubleRow: stationary [128, 2, 128] = two adjacent
    d-blocks of an S column-block, moving [128, 2, 512] = the matching two
    delta blocks, 0.5 cycles/row => PE ~25 us, fully DMA-bound.
    Yt[e,b] (e-block c) accumulates over j-pairs in PSUM [128,512] halves.
  - The same fp8 delta feeds the matmul and the final elementwise product,
    so q is the exact quadratic form of the perturbed inputs (error
    ~2 eps^T S delta ~ 0.06% measured, tolerance 2e-2).
  - Per column close: prod = deltaT_c .* Yt_c (DVE, bf16 out), then its
    row-sums accumulate into one persistent PSUM bank via a (2/16)-vector
    matmul (partitions 0/32 hold the two batch halves).
  - Tail: ACT sqrt with accum_out fuses sqrt+sum -> per-core [1,2] output.

Schedule: j-major over arriving delta blocks for the two BIG columns (14,15,
held in 4 PSUM banks); small columns 0..13 stream through the remaining
banks, paced to cover the late delta phase; S chunks preloaded so the x
stream is always the DMA tail; the last DMA (delta_15) unlocks only the
final DoubleRow pair of column 15.  Host: sum 16 half-sums / 8192.

Measured (hardware-loop amortized, 8 cores): ~89 us/iter vs ~198 us for the
bf16 dense-block baseline kernel -- 2.1x.  DMA floor for the 24.5 MiB at
measured ~320 GB/s effective is ~77 us.
"""

import numpy as np

P = 128
B_FULL, D = 8192, 2048
N_CORES = 8
B_SH = B_FULL // N_CORES    # 1024
NJ = D // P                 # 16 d/e blocks
NBLK = NJ * (NJ + 1) // 2   # 136 upper blocks
S_COLS = P * NBLK           # 17408
S_BOFF = [i * (i + 1) // 2 for i in range(NJ + 1)]  # block index offsets

BIG_COLS = [14, 15]
SMALL_COLS = [c for c in range(NJ) if c not in BIG_COLS]
N_SMALL_CELLS = sum(c + 1 for c in SMALL_COLS)   # 91
S_SCALE = 16.0

_CACHED = {}


def _build(b_sh=B_SH, d=D, loop=1):
    import contextlib

    import concourse.tile as tile
    from concourse import bacc, mybir

    nc = bacc.Bacc("TRN2", target_bir_lowering=False)
    f32 = mybir.dt.float32
    f32r = mybir.dt.float32r
    fp8 = mybir.dt.float8e4
    bf16 = mybir.dt.bfloat16
    DR = mybir.MatmulPerfMode.DoubleRow

    # [d-block, p, (orig|recon), batch-half, 512]
    x_t = nc.dram_tensor("x_t", [NJ, P, 2, 2, 512], f32,
                         kind="ExternalInput")
    s_pack = nc.dram_tensor("s_pack", [P, S_COLS], f32, kind="ExternalInput")
    q_out = nc.dram_tensor("q_out", [1, 2], f32, kind="ExternalOutput")

    with tile.TileContext(nc) as tc:
        with (
            tc.tile_pool(name="io", bufs=8) as io_pool,
            tc.tile_pool(name="sstage", bufs=3) as s_stage,
            tc.tile_pool(name="sbf", bufs=1) as s_pool,
            tc.tile_pool(name="dT", bufs=1) as dT_pool,
            tc.tile_pool(name="mkb", bufs=1) as mkb_pool,
            tc.tile_pool(name="mk", bufs=4) as mk_pool,
            tc.tile_pool(name="pr", bufs=4) as pr_pool,
            tc.tile_pool(name="accp", bufs=1) as acc_pool,
            tc.tile_pool(name="cst", bufs=1) as cst_pool,
            tc.tile_pool(name="tail", bufs=1) as tail_pool,
            tc.tile_pool(name="psq", bufs=1, space="PSUM") as psq_pool,
            tc.tile_pool(name="psbig", bufs=1, space="PSUM") as psb_pool,
            tc.tile_pool(name="pssm", bufs=3, space="PSUM") as pss_pool,
            tc.For_i(0, loop, 1) if loop > 1 else contextlib.nullcontext(),
        ):
            # --- constants ---
            tri = cst_pool.tile([P, P], f32, name="tri", tag="tri")
            nc.gpsimd.memset(tri[:], 1.0)
            nc.gpsimd.affine_select(
                out=tri[:], in_=tri[:], compare_op=mybir.AluOpType.is_ge,
                fill=0.5, base=-1, channel_multiplier=-1, pattern=[[1, P]])
            nc.gpsimd.affine_select(
                out=tri[:], in_=tri[:], compare_op=mybir.AluOpType.is_ge,
                fill=0.0, base=0, channel_multiplier=-1, pattern=[[1, P]])
            twos = cst_pool.tile([P, 1], bf16, name="twos", tag="twos")
            nc.vector.memset(twos[:], 2.0 / S_SCALE)

            # S blocks as [P, block, 128] fp8 (scaled by 16)
            warm = cst_pool.tile([1, 1], f32, name="warm", tag="warm")
            nc.scalar.sqrt(warm[:], tri[0:1, 0:1])

            s8 = s_pool.tile([P, NBLK, P], fp8, name="s8", tag="s8")
            qps2 = psq_pool.tile([64, 512], f32, name="qps2", tag="qps2")
            qps = [qps2[32 * h:32 * h + 1, :] for h in range(2)]
            # delta pair tiles [P, ko, h, 512]: ko = which block of the pair
            dpair = [dT_pool.tile([P, 2, 2, 512], fp8, name=f"dT_{m}",
                                  tag=f"dT_{m}") for m in range(NJ // 2)]
            masks = {}

            def emit_block(j):
                # one contiguous 1 MiB DMA per d-block (439 GB/s measured vs
                # 250 GB/s for strided half-loads), then per-half subs on two
                # engines
                xt = io_pool.tile([P, 2, 2, 512], f32, name=f"x_{j}",
                                  tag="io")
                nc.sync.dma_start(xt[:], x_t[j])
                # all subs on Pool so the io-buffer release stream never waits
                # on PE progress (prods/masks live on DVE); blocks 0 and 15
                # split across engines for startup/tail latency
                for h in range(2):
                    eng = (nc.vector if h == 0 and j in (0, NJ - 1)
                           else nc.gpsimd)
                    eng.tensor_sub(dpair[j // 2][:, j % 2, h, :],
                                   xt[:, 0, h, :], xt[:, 1, h, :])

            def emit_s_chunk(c, big=False):
                b0, b1 = S_BOFF[c], S_BOFF[c + 1]
                st = s_stage.tile([P, NJ, P], f32, name=f"sg_{c}", tag="sg")
                nc.sync.dma_start(st[:, 0:b1 - b0, :],
                                  s_pack[:, b0 * P:b1 * P])
                nc.scalar.mul(s8[:, b0:b1, :], st[:, 0:b1 - b0, :], S_SCALE)
                pool = mkb_pool if big else mk_pool
                # masked diag block (strict upper + diag/2); for odd c the
                # DoubleRow pair needs [block c-1 | masked c] side by side.
                nko = 2 if c % 2 == 1 else 1
                mk = pool.tile([P, nko, P], fp8, name=f"mk_{c}",
                               tag=f"mk_{c}" if big else "mk")
                if nko == 2:
                    nc.scalar.copy(mk[:, 0, :], s8[:, S_BOFF[c] + c - 1, :])
                nc.vector.tensor_tensor(
                    mk[:, nko - 1, :], s8[:, S_BOFF[c] + c, :], tri[:],
                    mybir.AluOpType.mult)
                masks[c] = mk

            def emit_cells(j0, c, ph):
                """pair-cell covering j0 (even) and j0+1 if <= c; or the
                masked single for even c."""
                first = (j0 == 0)
                if j0 + 1 <= c:   # DoubleRow pair (j0, j0+1)
                    last = (j0 + 1 == c)
                    lhsT = masks[c][:, :, :] if last else \
                        s8[:, S_BOFF[c] + j0:S_BOFF[c] + j0 + 2, :]
                    for h in range(2):
                        nc.tensor.matmul(
                            ph[h][:], lhsT, dpair[j0 // 2][:, :, h, :],
                            start=first, stop=last, perf_mode=DR)
                else:             # single masked diag (c even, j0 == c)
                    for h in range(2):
                        nc.tensor.matmul(
                            ph[h][:], masks[c][:, 0, :],
                            dpair[j0 // 2][:, j0 % 2, h, :],
                            start=first, stop=True)

            n_closed = [0]

            def emit_prod_acc(c, ph):
                # prod = delta_c .* Yt_c (DVE), then accumulate its row-sums
                # into the persistent q banks via a ones(=2/16)-matmul (PE).
                for h in range(2):
                    dlast = dpair[c // 2][:, c % 2, h, :]
                    prod = pr_pool.tile([P, 512], bf16,
                                        name=f"pr_{c}_{h}", tag="pr")
                    nc.vector.tensor_tensor(prod[:], ph[h][:], dlast,
                                            mybir.AluOpType.mult)
                    nc.tensor.matmul(qps[h], twos[:],
                                     prod[:],
                                     start=(n_closed[0] == 0),
                                     stop=(n_closed[0] == NJ - 1),
                                     skip_group_check=True)
                n_closed[0] += 1

            # --- schedule ---
            emit_block(0)
            emit_block(1)
            big_ph = {}
            for c in BIG_COLS:
                emit_s_chunk(c, big=True)
                big_ph[c] = [psb_pool.tile([P, 512], f32, name=f"psb_{c}_{h}",
                                           tag=f"psb_{c}_{h}")
                             for h in range(2)]
            emit_block(2)

            smalls = list(SMALL_COLS)
            to_load = list(SMALL_COLS)
            small_done = 0
            for j in range(NJ):
                if j + 3 < NJ:
                    emit_block(j + 3)
                # preload small-column S chunks well ahead of their cells so
                # the x blocks are always the stream's tail
                while to_load and to_load[0] <= j + 2:
                    emit_s_chunk(to_load.pop(0))
                for c in BIG_COLS:
                    if j <= c and (j % 2 == 1 or j == c):
                        emit_cells(j - 1 if j % 2 == 1 else j, c, big_ph[c])
                        if j == c:
                            emit_prod_acc(c, big_ph[c])
                cap = (N_SMALL_CELLS * (j + 1) + 11) // 12
                while smalls and smalls[0] <= j and \
                        small_done + smalls[0] + 1 <= cap:
                    c = smalls.pop(0)
                    ph = [pss_pool.tile([P, 512], f32, name=f"ps_{c}_{h}",
                                        tag="ps") for h in range(2)]
                    for j0 in range(0, c + 1, 2):
                        emit_cells(j0, c, ph)
                    emit_prod_acc(c, ph)
                    small_done += c + 1

            # --- tail: q is already in qps; fused sqrt+sum per half ---
            red = tail_pool.tile([1, 2], f32, name="red", tag="red")
            sq = tail_pool.tile([1, b_sh], f32, name="sq", tag="sq")
            for h in range(2):
                nc.scalar.activation(
                    out=sq[:, h * 512:(h + 1) * 512], in_=qps[h],
                    func=mybir.ActivationFunctionType.Sqrt,
                    accum_out=red[:, h:h + 1])
            nc.sync.dma_start(q_out[:], red[:])

    nc.compile()
    return nc


def _get_nc():
    if "nc" not in _CACHED:
        _CACHED["nc"] = _build()
    return _CACHED["nc"]


def make_in_maps(original, reconstruction, S_inv):
    """Host-side sharding/packing (pure slicing + layout rearrangement)."""
    s = np.asarray(S_inv, dtype=np.float32)
    s_pack = np.ascontiguousarray(np.concatenate(
        [s[j * P:(j + 1) * P, i * P:(i + 1) * P]
         for i in range(NJ) for j in range(i + 1)], axis=1))

    in_maps = []
    for i in range(N_CORES):
        sl = slice(i * B_SH, (i + 1) * B_SH)
        x = np.empty((D, 2 * B_SH), np.float32)
        x[:, 0:B_SH] = np.asarray(original[sl], np.float32).T
        x[:, B_SH:] = np.asarray(reconstruction[sl], np.float32).T
        in_maps.append({"x_t": x.reshape(NJ, P, 2, 2, 512), "s_pack": s_pack})
    return in_maps


def kernel(original: np.ndarray, reconstruction: np.ndarray,
           S_inv: np.ndarray) -> np.ndarray:
    from concourse import bass_utils

    nc = _get_nc()
    in_maps = make_in_maps(original, reconstruction, S_inv)
    res = bass_utils.run_bass_kernel_spmd(
        nc, in_maps, core_ids=list(range(N_CORES)),
        trace=_CACHED.get("trace", False),
    )
    _CACHED["last_results"] = res

    total = sum(float(np.asarray(r["q_out"]).sum()) for r in res.results)
    return np.float32(total / B_FULL)


# revision 5
# speedup vs baseline: 1.7361x; 1.0008x over previous
"""Trainium2 Bass kernel for the Mahalanobis loss:

    out = mean_b( sqrt( delta[b] @ S_inv @ delta[b] ) ),  delta = original - reconstruction

Full shapes: original/reconstruction [8192, 2048] f32, S_inv [2048, 2048] f32.
Data-parallel over batch on 8 NeuronCores: core i handles rows [i*1024,(i+1)*1024).

Math: S_inv is symmetric, so  q[b] = 2 * delta[b] @ U' @ delta[b]  with
U' = strict_upper(S) + diag(S)/2.  Only the 136 upper-triangular 128x128
blocks of S are uploaded (8.5 MiB instead of 16) and matmul'd (53% of the
dense FLOPs).  Per-core HBM traffic: 16 MiB x (orig|recon transposed,
block-packed) + 8.5 MiB S-upper = 24.5 MiB, the binding resource.

Device kernel (per core):
  - x_t [16,128,2,2,512] f32: host-transposed orig/recon packed so each
    d-block is one contiguous 1 MiB DMA (measured ~1.4x faster than strided
    half-loads).  deltaT = orig - recon computed per batch-half straight to
    fp8e4 on DVE (h0) and Pool (h1) -- delta in [d, b] layout, no on-device
    transpose.
  - S blocks cast f32 -> fp8e4 with scale 16 on ACT (off-diag ~N(0,1/45)
    would sit in fp8 subnormal range unscaled); 1/16 folded into the final
    reduction vector.  Diagonal blocks masked on-device (strict upper +
    diag/2) via gpsimd affine_select triangle mask.
  - Matmuls in FP8 DoubleRow: stationary [128, 2, 128] = two adjacent
    d-blocks of an S column-block, moving [128, 2, 512] = the matching two
    delta blocks, 0.5 cycles/row => PE ~25 us, fully DMA-bound.
    Yt[e,b] (e-block c) accumulates over j-pairs in PSUM [128,512] halves.
  - The same fp8 delta feeds the matmul and the final elementwise product,
    so q is the exact quadratic form of the perturbed inputs (error
    ~2 eps^T S delta ~ 0.06% measured, tolerance 2e-2).
  - Per column close: prod = deltaT_c .* Yt_c (DVE, bf16 out), then its
    row-sums accumulate into one persistent PSUM bank via a (2/16)-vector
    matmul (partitions 0/32 hold the two batch halves).
  - Tail: ACT sqrt with accum_out fuses sqrt+sum -> per-core [1,2] output.

Schedule: j-major over arriving delta blocks for the two BIG columns (14,15,
held in 4 PSUM banks); small columns 0..13 stream through the remaining
banks, paced to cover the late delta phase; S chunks preloaded so the x
stream is always the DMA tail; the last DMA (delta_15) unlocks only the
final DoubleRow pair of column 15.  Host: sum 16 half-sums / 8192.

Measured (hardware-loop amortized, 8 cores): ~88 us/iter vs ~198 us for the
bf16 dense-block baseline kernel -- 2.1x.  DMA floor for the 24.5 MiB at
measured ~320 GB/s effective is ~77 us.
"""

import numpy as np

P = 128
B_FULL, D = 8192, 2048
N_CORES = 8
B_SH = B_FULL // N_CORES    # 1024
NJ = D // P                 # 16 d/e blocks
NBLK = NJ * (NJ + 1) // 2   # 136 upper blocks
S_COLS = P * NBLK           # 17408
S_BOFF = [i * (i + 1) // 2 for i in range(NJ + 1)]  # block index offsets

BIG_COLS = [14, 15]
SMALL_COLS = [c for c in range(NJ) if c not in BIG_COLS]
N_SMALL_CELLS = sum(c + 1 for c in SMALL_COLS)   # 91
S_SCALE = 16.0

_CACHED = {}


def _build(b_sh=B_SH, d=D, loop=1):
    import contextlib

    import concourse.tile as tile
    from concourse import bacc, mybir

    nc = bacc.Bacc("TRN2", target_bir_lowering=False)
    f32 = mybir.dt.float32
    f32r = mybir.dt.float32r
    fp8 = mybir.dt.float8e4
    bf16 = mybir.dt.bfloat16
    DR = mybir.MatmulPerfMode.DoubleRow

    # [d-block, p, (orig|recon), batch-half, 512]
    x_t = nc.dram_tensor("x_t", [NJ, P, 2, 2, 512], f32,
                         kind="ExternalInput")
    s_pack = nc.dram_tensor("s_pack", [P, S_COLS], f32, kind="ExternalInput")
    q_out = nc.dram_tensor("q_out", [1, 2], f32, kind="ExternalOutput")

    with tile.TileContext(nc) as tc:
        with (
            tc.tile_pool(name="io", bufs=8) as io_pool,
            tc.tile_pool(name="sstage", bufs=3) as s_stage,
            tc.tile_pool(name="sbf", bufs=1) as s_pool,
            tc.tile_pool(name="dT", bufs=1) as dT_pool,
            tc.tile_pool(name="mkb", bufs=1) as mkb_pool,
            tc.tile_pool(name="mk", bufs=4) as mk_pool,
            tc.tile_pool(name="pr", bufs=4) as pr_pool,
            tc.tile_pool(name="accp", bufs=1) as acc_pool,
            tc.tile_pool(name="cst", bufs=1) as cst_pool,
            tc.tile_pool(name="tail", bufs=1) as tail_pool,
            tc.tile_pool(name="psq", bufs=1, space="PSUM") as psq_pool,
            tc.tile_pool(name="psbig", bufs=1, space="PSUM") as psb_pool,
            tc.tile_pool(name="pssm", bufs=3, space="PSUM") as pss_pool,
            tc.For_i(0, loop, 1) if loop > 1 else contextlib.nullcontext(),
        ):
            # --- constants ---
            tri = cst_pool.tile([P, P], f32, name="tri", tag="tri")
            nc.gpsimd.memset(tri[:], 1.0)
            nc.gpsimd.affine_select(
                out=tri[:], in_=tri[:], compare_op=mybir.AluOpType.is_ge,
                fill=0.5, base=-1, channel_multiplier=-1, pattern=[[1, P]])
            nc.gpsimd.affine_select(
                out=tri[:], in_=tri[:], compare_op=mybir.AluOpType.is_ge,
                fill=0.0, base=0, channel_multiplier=-1, pattern=[[1, P]])
            twos = cst_pool.tile([P, 1], bf16, name="twos", tag="twos")
            nc.vector.memset(twos[:], 2.0 / S_SCALE)

            # S blocks as [P, block, 128] fp8 (scaled by 16)
            warm = cst_pool.tile([1, 1], f32, name="warm", tag="warm")
            nc.scalar.sqrt(warm[:], tri[0:1, 0:1])

            s8 = s_pool.tile([P, NBLK, P], fp8, name="s8", tag="s8")
            qps2 = psq_pool.tile([64, 512], f32, name="qps2", tag="qps2")
            qps = [qps2[32 * h:32 * h + 1, :] for h in range(2)]
            # delta pair tiles [P, ko, h, 512]: ko = which block of the pair
            dpair = [dT_pool.tile([P, 2, 2, 512], fp8, name=f"dT_{m}",
                                  tag=f"dT_{m}") for m in range(NJ // 2)]
            masks = {}

            def emit_block(j):
                # one contiguous 1 MiB DMA per d-block (439 GB/s measured vs
                # 250 GB/s for strided half-loads), then per-half subs on two
                # engines
                xt = io_pool.tile([P, 2, 2, 512], f32, name=f"x_{j}",
                                  tag="io")
                nc.sync.dma_start(xt[:], x_t[j])
                # all subs on Pool so the io-buffer release stream never waits
                # on PE progress (prods/masks live on DVE); blocks 0 and 15
                # split across engines for startup/tail latency
                for h in range(2):
                    eng = (nc.vector if h == 0 and j in (0, NJ - 1)
                           else nc.gpsimd)
                    eng.tensor_sub(dpair[j // 2][:, j % 2, h, :],
                                   xt[:, 0, h, :], xt[:, 1, h, :])

            def emit_s_chunk(c, big=False):
                b0, b1 = S_BOFF[c], S_BOFF[c + 1]
                st = s_stage.tile([P, NJ, P], f32, name=f"sg_{c}", tag="sg")
                nc.sync.dma_start(st[:, 0:b1 - b0, :],
                                  s_pack[:, b0 * P:b1 * P])
                nc.scalar.mul(s8[:, b0:b1, :], st[:, 0:b1 - b0, :], S_SCALE)
                pool = mkb_pool if big else mk_pool
                # masked diag block (strict upper + diag/2); for odd c the
                # DoubleRow pair needs [block c-1 | masked c] side by side.
                nko = 2 if c % 2 == 1 else 1
                mk = pool.tile([P, nko, P], fp8, name=f"mk_{c}",
                               tag=f"mk_{c}" if big else "mk")
                if nko == 2:
                    nc.scalar.copy(mk[:, 0, :], s8[:, S_BOFF[c] + c - 1, :])
                nc.vector.tensor_tensor(
                    mk[:, nko - 1, :], s8[:, S_BOFF[c] + c, :], tri[:],
                    mybir.AluOpType.mult)
                masks[c] = mk

            def emit_cells(j0, c, ph):
                """pair-cell covering j0 (even) and j0+1 if <= c; or the
                masked single for even c."""
                first = (j0 == 0)
                if j0 + 1 <= c:   # DoubleRow pair (j0, j0+1)
                    last = (j0 + 1 == c)
                    lhsT = masks[c][:, :, :] if last else \
                        s8[:, S_BOFF[c] + j0:S_BOFF[c] + j0 + 2, :]
                    for h in range(2):
                        nc.tensor.matmul(
                            ph[h][:], lhsT, dpair[j0 // 2][:, :, h, :],
                            start=first, stop=last, perf_mode=DR)
                else:             # single masked diag (c even, j0 == c)
                    for h in range(2):
                        nc.tensor.matmul(
                            ph[h][:], masks[c][:, 0, :],
                            dpair[j0 // 2][:, j0 % 2, h, :],
                            start=first, stop=True)

            n_closed = [0]

            def emit_prod_acc(c, ph):
                # prod = delta_c .* Yt_c (DVE), then accumulate its row-sums
                # into the persistent q banks via a ones(=2/16)-matmul (PE).
                for h in range(2):
                    dlast = dpair[c // 2][:, c % 2, h, :]
                    prod = pr_pool.tile([P, 512], bf16,
                                        name=f"pr_{c}_{h}", tag="pr")
                    nc.vector.tensor_tensor(prod[:], ph[h][:], dlast,
                                            mybir.AluOpType.mult)
                    nc.tensor.matmul(qps[h], twos[:],
                                     prod[:],
                                     start=(n_closed[0] == 0),
                                     stop=(n_closed[0] == NJ - 1),
                                     skip_group_check=True)
                n_closed[0] += 1

            # --- schedule ---
            emit_block(0)
            emit_block(1)
            big_ph = {}
            for c in BIG_COLS:
                emit_s_chunk(c, big=True)
                big_ph[c] = [psb_pool.tile([P, 512], f32, name=f"psb_{c}_{h}",
                                           tag=f"psb_{c}_{h}")
                             for h in range(2)]
            emit_block(2)

            smalls = list(SMALL_COLS)
            to_load = list(SMALL_COLS)
            small_done = 0
            for j in range(NJ):
                if j + 3 < NJ:
                    emit_block(j + 3)
                # preload small-column S chunks well ahead of their cells so
                # the x blocks are always the stream's tail
                while to_load and to_load[0] <= j + 2:
                    emit_s_chunk(to_load.pop(0))
                for c in BIG_COLS:
                    if j <= c and (j % 2 == 1 or j == c):
                        emit_cells(j - 1 if j % 2 == 1 else j, c, big_ph[c])
                        if j == c:
                            emit_prod_acc(c, big_ph[c])
                cap = (N_SMALL_CELLS * (j + 1) + 11) // 12
                while smalls and smalls[0] <= j and \
                        small_done + smalls[0] + 1 <= cap:
                    c = smalls.pop(0)
                    ph = [pss_pool.tile([P, 512], f32, name=f"ps_{c}_{h}",
                                        tag="ps") for h in range(2)]
                    for j0 in range(0, c + 1, 2):
                        emit_cells(j0, c, ph)
                    emit_prod_acc(c, ph)
                    small_done += c + 1

            # --- tail: q is already in qps; fused sqrt+sum per half ---
            red = tail_pool.tile([1, 2], f32, name="red", tag="red")
            sq = tail_pool.tile([1, b_sh], f32, name="sq", tag="sq")
            for h in range(2):
                nc.scalar.activation(
                    out=sq[:, h * 512:(h + 1) * 512], in_=qps[h],
                    func=mybir.ActivationFunctionType.Sqrt,
                    accum_out=red[:, h:h + 1])
            # out-DMA on the ACT queue: it follows the sqrts in-order there,
            # so the SP queue never blocks on the tail and the next For_i
            # iteration's x-loads issue immediately (tail hides under them)
            nc.scalar.dma_start(q_out[:], red[:])

    nc.compile()
    return nc


def _get_nc():
    if "nc" not in _CACHED:
        _CACHED["nc"] = _build()
    return _CACHED["nc"]


def make_in_maps(original, reconstruction, S_inv):
    """Host-side sharding/packing (pure slicing + layout rearrangement)."""
    s = np.asarray(S_inv, dtype=np.float32)
    s_pack = np.ascontiguousarray(np.concatenate(
        [s[j * P:(j + 1) * P, i * P:(i + 1) * P]
         for i in range(NJ) for j in range(i + 1)], axis=1))

    in_maps = []
    for i in range(N_CORES):
        sl = slice(i * B_SH, (i + 1) * B_SH)
        x = np.empty((D, 2 * B_SH), np.float32)
        x[:, 0:B_SH] = np.asarray(original[sl], np.float32).T
        x[:, B_SH:] = np.asarray(reconstruction[sl], np.float32).T
        in_maps.append({"x_t": x.reshape(NJ, P, 2, 2, 512), "s_pack": s_pack})
    return in_maps


def kernel(original: np.ndarray, reconstruction: np.ndarray,
           S_inv: np.ndarray) -> np.ndarray:
    from concourse import bass_utils

    nc = _get_nc()
    in_maps = make_in_maps(original, reconstruction, S_inv)
    res = bass_utils.run_bass_kernel_spmd(
        nc, in_maps, core_ids=list(range(N_CORES)),
        trace=_CACHED.get("trace", False),
    )
    _CACHED["last_results"] = res

    total = sum(float(np.asarray(r["q_out"]).sum()) for r in res.results)
    return np.float32(total / B_FULL)


# revision 6
# speedup vs baseline: 1.7792x; 1.0248x over previous
"""Trainium2 Bass kernel for the Mahalanobis loss:

    out = mean_b( sqrt( delta[b] @ S_inv @ delta[b] ) ),  delta = original - reconstruction

Full shapes: original/reconstruction [8192, 2048] f32, S_inv [2048, 2048] f32.
Data-parallel over batch on 8 NeuronCores: core i handles rows [i*1024,(i+1)*1024).

Math: S_inv is symmetric, so  q[b] = 2 * delta[b] @ U' @ delta[b]  with
U' = strict_upper(S) + diag(S)/2.  Only the 136 upper-triangular 128x128
blocks of S are uploaded (8.5 MiB instead of 16) and matmul'd (53% of the
dense FLOPs).  Per-core HBM traffic: 16 MiB x (orig|recon transposed,
block-packed) + 8.5 MiB S-upper = 24.5 MiB, the binding resource.

Device kernel (per core):
  - x_t [16,128,2,2,512] f32: host-transposed orig/recon packed so each
    d-block is one contiguous 1 MiB DMA (measured ~1.4x faster than strided
    half-loads).  deltaT = orig - recon computed per batch-half straight to
    fp8e4 on DVE (h0) and Pool (h1) -- delta in [d, b] layout, no on-device
    transpose.
  - S blocks cast f32 -> fp8e4 with scale 16 on ACT (off-diag ~N(0,1/45)
    would sit in fp8 subnormal range unscaled); 1/16 folded into the final
    reduction vector.  Diagonal blocks masked on-device (strict upper +
    diag/2) via gpsimd affine_select triangle mask.
  - Matmuls in FP8 DoubleRow: stationary [128, 2, 128] = two adjacent
    d-blocks of an S column-block, moving [128, 2, 512] = the matching two
    delta blocks, 0.5 cycles/row => PE ~25 us, fully DMA-bound.
    Yt[e,b] (e-block c) accumulates over j-pairs in PSUM [128,512] halves.
  - The same fp8 delta feeds the matmul and the final elementwise product,
    so q is the exact quadratic form of the perturbed inputs (error
    ~2 eps^T S delta ~ 0.06% measured, tolerance 2e-2).
  - Per column close: prod = deltaT_c .* Yt_c (DVE, bf16 out), then its
    row-sums accumulate into one persistent PSUM bank via a (2/16)-vector
    matmul (partitions 0/32 hold the two batch halves).
  - Tail: ACT sqrt with accum_out fuses sqrt+sum -> per-core [1,2] output.

Schedule: j-major over arriving delta blocks for the two BIG columns (14,15,
held in 4 PSUM banks); small columns 0..13 stream through the remaining
banks, paced to cover the late delta phase; S chunks preloaded so the x
stream is always the DMA tail; the last DMA (delta_15) unlocks only the
final DoubleRow pair of column 15.  Host: sum 16 half-sums / 8192.

Measured (hardware-loop amortized, 8 cores): ~88 us/iter (v12: per-chunk contiguous S tensors) vs ~198 us for the
bf16 dense-block baseline kernel -- 2.1x.  DMA floor for the 24.5 MiB at
measured ~320 GB/s effective is ~77 us.
"""

import numpy as np

P = 128
B_FULL, D = 8192, 2048
N_CORES = 8
B_SH = B_FULL // N_CORES    # 1024
NJ = D // P                 # 16 d/e blocks
NBLK = NJ * (NJ + 1) // 2   # 136 upper blocks
S_COLS = P * NBLK           # 17408
S_BOFF = [i * (i + 1) // 2 for i in range(NJ + 1)]  # block index offsets

BIG_COLS = [14, 15]
SMALL_COLS = [c for c in range(NJ) if c not in BIG_COLS]
N_SMALL_CELLS = sum(c + 1 for c in SMALL_COLS)   # 91
S_SCALE = 16.0

_CACHED = {}


def _build(b_sh=B_SH, d=D, loop=1):
    import contextlib

    import concourse.tile as tile
    from concourse import bacc, mybir

    nc = bacc.Bacc("TRN2", target_bir_lowering=False)
    f32 = mybir.dt.float32
    f32r = mybir.dt.float32r
    fp8 = mybir.dt.float8e4
    bf16 = mybir.dt.bfloat16
    DR = mybir.MatmulPerfMode.DoubleRow

    # [d-block, p, (orig|recon), batch-half, 512]
    x_t = nc.dram_tensor("x_t", [NJ, P, 2, 2, 512], f32,
                         kind="ExternalInput")
    # one contiguous DRAM tensor per S column-chunk (strided slices of a
    # single packed tensor measured ~15-25% slower DMA)
    s_cs = [nc.dram_tensor(f"s_c{c}", [P, (c + 1) * P], f32,
                           kind="ExternalInput") for c in range(NJ)]
    q_out = nc.dram_tensor("q_out", [1, 2], f32, kind="ExternalOutput")

    with tile.TileContext(nc) as tc:
        with (
            tc.tile_pool(name="io", bufs=8) as io_pool,
            tc.tile_pool(name="sstage", bufs=3) as s_stage,
            tc.tile_pool(name="sbf", bufs=1) as s_pool,
            tc.tile_pool(name="dT", bufs=1) as dT_pool,
            tc.tile_pool(name="mkb", bufs=1) as mkb_pool,
            tc.tile_pool(name="mk", bufs=4) as mk_pool,
            tc.tile_pool(name="pr", bufs=4) as pr_pool,
            tc.tile_pool(name="accp", bufs=1) as acc_pool,
            tc.tile_pool(name="cst", bufs=1) as cst_pool,
            tc.tile_pool(name="tail", bufs=1) as tail_pool,
            tc.tile_pool(name="psq", bufs=1, space="PSUM") as psq_pool,
            tc.tile_pool(name="psbig", bufs=1, space="PSUM") as psb_pool,
            tc.tile_pool(name="pssm", bufs=3, space="PSUM") as pss_pool,
            tc.For_i(0, loop, 1) if loop > 1 else contextlib.nullcontext(),
        ):
            # --- constants ---
            tri = cst_pool.tile([P, P], f32, name="tri", tag="tri")
            nc.gpsimd.memset(tri[:], 1.0)
            nc.gpsimd.affine_select(
                out=tri[:], in_=tri[:], compare_op=mybir.AluOpType.is_ge,
                fill=0.5, base=-1, channel_multiplier=-1, pattern=[[1, P]])
            nc.gpsimd.affine_select(
                out=tri[:], in_=tri[:], compare_op=mybir.AluOpType.is_ge,
                fill=0.0, base=0, channel_multiplier=-1, pattern=[[1, P]])
            twos = cst_pool.tile([P, 1], bf16, name="twos", tag="twos")
            nc.vector.memset(twos[:], 2.0 / S_SCALE)

            # S blocks as [P, block, 128] fp8 (scaled by 16)
            warm = cst_pool.tile([1, 1], f32, name="warm", tag="warm")
            nc.scalar.sqrt(warm[:], tri[0:1, 0:1])

            s8 = s_pool.tile([P, NBLK, P], fp8, name="s8", tag="s8")
            qps2 = psq_pool.tile([64, 512], f32, name="qps2", tag="qps2")
            qps = [qps2[32 * h:32 * h + 1, :] for h in range(2)]
            # delta pair tiles [P, ko, h, 512]: ko = which block of the pair
            dpair = [dT_pool.tile([P, 2, 2, 512], fp8, name=f"dT_{m}",
                                  tag=f"dT_{m}") for m in range(NJ // 2)]
            masks = {}

            def emit_block(j):
                # one contiguous 1 MiB DMA per d-block (439 GB/s measured vs
                # 250 GB/s for strided half-loads), then per-half subs on two
                # engines
                xt = io_pool.tile([P, 2, 2, 512], f32, name=f"x_{j}",
                                  tag="io")
                nc.sync.dma_start(xt[:], x_t[j])
                # all subs on Pool so the io-buffer release stream never waits
                # on PE progress (prods/masks live on DVE); blocks 0 and 15
                # split across engines for startup/tail latency
                for h in range(2):
                    eng = (nc.vector if h == 0 and j in (0, NJ - 1)
                           else nc.gpsimd)
                    eng.tensor_sub(dpair[j // 2][:, j % 2, h, :],
                                   xt[:, 0, h, :], xt[:, 1, h, :])

            def emit_s_chunk(c, big=False):
                b0, b1 = S_BOFF[c], S_BOFF[c + 1]
                st = s_stage.tile([P, NJ, P], f32, name=f"sg_{c}", tag="sg")
                nc.sync.dma_start(st[:, 0:b1 - b0, :], s_cs[c][:])
                nc.scalar.mul(s8[:, b0:b1, :], st[:, 0:b1 - b0, :], S_SCALE)
                pool = mkb_pool if big else mk_pool
                # masked diag block (strict upper + diag/2); for odd c the
                # DoubleRow pair needs [block c-1 | masked c] side by side.
                nko = 2 if c % 2 == 1 else 1
                mk = pool.tile([P, nko, P], fp8, name=f"mk_{c}",
                               tag=f"mk_{c}" if big else "mk")
                if nko == 2:
                    nc.scalar.copy(mk[:, 0, :], s8[:, S_BOFF[c] + c - 1, :])
                nc.vector.tensor_tensor(
                    mk[:, nko - 1, :], s8[:, S_BOFF[c] + c, :], tri[:],
                    mybir.AluOpType.mult)
                masks[c] = mk

            def emit_cells(j0, c, ph):
                """pair-cell covering j0 (even) and j0+1 if <= c; or the
                masked single for even c."""
                first = (j0 == 0)
                if j0 + 1 <= c:   # DoubleRow pair (j0, j0+1)
                    last = (j0 + 1 == c)
                    lhsT = masks[c][:, :, :] if last else \
                        s8[:, S_BOFF[c] + j0:S_BOFF[c] + j0 + 2, :]
                    for h in range(2):
                        nc.tensor.matmul(
                            ph[h][:], lhsT, dpair[j0 // 2][:, :, h, :],
                            start=first, stop=last, perf_mode=DR)
                else:             # single masked diag (c even, j0 == c)
                    for h in range(2):
                        nc.tensor.matmul(
                            ph[h][:], masks[c][:, 0, :],
                            dpair[j0 // 2][:, j0 % 2, h, :],
                            start=first, stop=True)

            n_closed = [0]

            def emit_prod_acc(c, ph):
                # prod = delta_c .* Yt_c (DVE), then accumulate its row-sums
                # into the persistent q banks via a ones(=2/16)-matmul (PE).
                for h in range(2):
                    dlast = dpair[c // 2][:, c % 2, h, :]
                    prod = pr_pool.tile([P, 512], bf16,
                                        name=f"pr_{c}_{h}", tag="pr")
                    nc.vector.tensor_tensor(prod[:], ph[h][:], dlast,
                                            mybir.AluOpType.mult)
                    nc.tensor.matmul(qps[h], twos[:],
                                     prod[:],
                                     start=(n_closed[0] == 0),
                                     stop=(n_closed[0] == NJ - 1),
                                     skip_group_check=True)
                n_closed[0] += 1

            # --- schedule ---
            emit_block(0)
            emit_block(1)
            big_ph = {}
            for c in BIG_COLS:
                emit_s_chunk(c, big=True)
                big_ph[c] = [psb_pool.tile([P, 512], f32, name=f"psb_{c}_{h}",
                                           tag=f"psb_{c}_{h}")
                             for h in range(2)]
            emit_block(2)

            smalls = list(SMALL_COLS)
            to_load = list(SMALL_COLS)
            small_done = 0
            for j in range(NJ):
                if j + 3 < NJ:
                    emit_block(j + 3)
                # preload small-column S chunks well ahead of their cells so
                # the x blocks are always the stream's tail
                while to_load and to_load[0] <= j + 2:
                    emit_s_chunk(to_load.pop(0))
                for c in BIG_COLS:
                    if j <= c and (j % 2 == 1 or j == c):
                        emit_cells(j - 1 if j % 2 == 1 else j, c, big_ph[c])
                        if j == c:
                            emit_prod_acc(c, big_ph[c])
                cap = (N_SMALL_CELLS * (j + 1) + 11) // 12
                while smalls and smalls[0] <= j and \
                        small_done + smalls[0] + 1 <= cap:
                    c = smalls.pop(0)
                    ph = [pss_pool.tile([P, 512], f32, name=f"ps_{c}_{h}",
                                        tag="ps") for h in range(2)]
                    for j0 in range(0, c + 1, 2):
                        emit_cells(j0, c, ph)
                    emit_prod_acc(c, ph)
                    small_done += c + 1

            # --- tail: q is already in qps; fused sqrt+sum per half ---
            red = tail_pool.tile([1, 2], f32, name="red", tag="red")
            sq = tail_pool.tile([1, b_sh], f32, name="sq", tag="sq")
            for h in range(2):
                nc.scalar.activation(
                    out=sq[:, h * 512:(h + 1) * 512], in_=qps[h],
                    func=mybir.ActivationFunctionType.Sqrt,
                    accum_out=red[:, h:h + 1])
            # out-DMA on the ACT queue: it follows the sqrts in-order there,
            # so the SP queue never blocks on the tail and the next For_i
            # iteration's x-loads issue immediately (tail hides under them)
            nc.scalar.dma_start(q_out[:], red[:])

    nc.compile()
    return nc


def _get_nc():
    if "nc" not in _CACHED:
        _CACHED["nc"] = _build()
    return _CACHED["nc"]


def make_in_maps(original, reconstruction, S_inv):
    """Host-side sharding/packing (pure slicing + layout rearrangement)."""
    s = np.asarray(S_inv, dtype=np.float32)
    s_chunks = {
        f"s_c{c}": np.ascontiguousarray(np.concatenate(
            [s[j * P:(j + 1) * P, c * P:(c + 1) * P] for j in range(c + 1)],
            axis=1))
        for c in range(NJ)}

    in_maps = []
    for i in range(N_CORES):
        sl = slice(i * B_SH, (i + 1) * B_SH)
        x = np.empty((D, 2 * B_SH), np.float32)
        x[:, 0:B_SH] = np.asarray(original[sl], np.float32).T
        x[:, B_SH:] = np.asarray(reconstruction[sl], np.float32).T
        in_maps.append({"x_t": x.reshape(NJ, P, 2, 2, 512), **s_chunks})
    return in_maps


def kernel(original: np.ndarray, reconstruction: np.ndarray,
           S_inv: np.ndarray) -> np.ndarray:
    from concourse import bass_utils

    nc = _get_nc()
    in_maps = make_in_maps(original, reconstruction, S_inv)
    res = bass_utils.run_bass_kernel_spmd(
        nc, in_maps, core_ids=list(range(N_CORES)),
        trace=_CACHED.get("trace", False),
    )
    _CACHED["last_results"] = res

    total = sum(float(np.asarray(r["q_out"]).sum()) for r in res.results)
    return np.float32(total / B_FULL)
